# revision 5
# baseline (speedup 1.0000x reference)
"""Trainium2 Bass kernel for nn_CrossAttention_38723425140909 (SACFA sparse cross-attention).

Problem (hardcoded):
  x [16, 640, 640] f32, Wq/Wk/Wv/Wo [640, 640], bo [640],
  sacfa_mask [10240] with n_sel=2048 selected tokens.
  out = attention(q=xWq, kv=[frame kv | gathered SACFA kv]) Wo + bo.

Sharding: B=16 frames data-parallel over 8 cores (2 frames/core).  The
gathered SACFA tokens are needed by every frame, so instead of an
on-device all-gather each core receives the (host-gathered) selected
input rows x_sel = x_flat[sel] and redundantly projects k_tok/v_tok
locally -- fully SPMD, no collectives.

Device layout choices (all matmuls bf16, fp32 PSUM accumulation):
  - host pre-transposes x slices to xT [C, tok] so projections produce
    qT/kT in [d, tok] layout directly (lhsT = W tiles) and v in
    [tok, c] layout (lhsT = xT tiles).
  - scores are computed TRANSPOSED: sT[kv, tok] = kT_h.T-slice @ qT_h,
    so exp(sT) can feed the PV matmul as the moving operand with
    v stationary: outT_h[d+1, tok] = [v_h | 1].T @ exp(sT).
    The appended ones-column of v gives the softmax denominator as
    row d of the PV result -- no separate reduction pass.
  - reciprocal of the denominator row is partition-broadcast (DMA) and
    applied with one vector multiply; normalized per-head outputs feed
    the final projection as 8 accumulating K=80 matmuls per c-tile.
  - softmax max-subtraction is skipped: scores are ~N(0,1) (max |s| < 8
    for these inputs), safely inside fp32 exp range.
"""

import numpy as np
import ml_dtypes

P = 128
B, N, C, H = 16, 640, 640, 8
D = C // H            # 80
NSEL = 2048
NCORES = 8
BL = B // NCORES      # 2 frames per core
TOK = BL * N          # 1280 local query tokens
KC = C // P           # 5 contraction tiles
NKO = N // P          # 5 own-kv tiles per frame
NKS = NSEL // P       # 16 shared-kv tiles
NKV = NKO + NKS       # 21 kv tiles per (frame, head)
DA = 97               # head dim (80) + zero pad to 96 + denominator row
DNM = 96              # 32-aligned denominator row (BIR partition-base rule)

_BF16 = ml_dtypes.bfloat16


def _build_bass():
    import concourse.bacc as bacc
    import concourse.tile as tile
    from concourse import mybir

    bf16 = mybir.dt.bfloat16
    f32 = mybir.dt.float32

    nc = bacc.Bacc(
        "TRN2",
        target_bir_lowering=False,
        debug=False,
        enable_asserts=False,
        num_devices=NCORES,
    )

    xt = nc.dram_tensor("xt", [C, TOK], bf16, kind="ExternalInput")
    xst = nc.dram_tensor("xst", [C, NSEL], bf16, kind="ExternalInput")
    wq = nc.dram_tensor("wq", [C, C], bf16, kind="ExternalInput")
    wk = nc.dram_tensor("wk", [C, C], bf16, kind="ExternalInput")
    wv = nc.dram_tensor("wv", [C, C], bf16, kind="ExternalInput")
    wo = nc.dram_tensor("wo", [C, C], bf16, kind="ExternalInput")
    bo = nc.dram_tensor("bo", [C], f32, kind="ExternalInput")
    outt = nc.dram_tensor("outt", [C, TOK], f32, kind="ExternalOutput")

    with tile.TileContext(nc) as tc:
        _body(tc, mybir, xt, xst, wq, wk, wv, wo, bo, outt)

    nc.compile()
    return nc


def _body(tc, mybir, xt, xst, wq, wk, wv, wo, bo, outt):
    nc = tc.nc
    bf16 = mybir.dt.bfloat16
    f32 = mybir.dt.float32
    Exp = mybir.ActivationFunctionType.Exp

    with (
        tc.tile_pool(name="singles", bufs=1) as singles,
        tc.tile_pool(name="ppsum", bufs=2, space="PSUM") as ppsum,
        tc.tile_pool(name="qpsum", bufs=2, space="PSUM") as qpsum,
        tc.tile_pool(name="expp", bufs=4) as expp,
        tc.tile_pool(name="rp", bufs=3) as rp,
        tc.tile_pool(name="ob", bufs=3) as ob,
    ):
        # ---- load inputs ----
        xt_sb = singles.tile([P, KC, TOK], bf16)
        nc.sync.dma_start(xt_sb, xt.ap().rearrange("(k p) t -> p k t", p=P))
        xst_sb = singles.tile([P, KC, NSEL], bf16, tag="xst_nrm")
        nc.sync.dma_start(xst_sb, xst.ap().rearrange("(k p) t -> p k t", p=P))
        wq_sb = singles.tile([P, KC, C], bf16)
        nc.sync.dma_start(wq_sb, wq.ap().rearrange("(k p) n -> p k n", p=P))
        wk_sb = singles.tile([P, KC, C], bf16)
        nc.sync.dma_start(wk_sb, wk.ap().rearrange("(k p) n -> p k n", p=P))
        wv_sb = singles.tile([P, KC, C], bf16)
        nc.sync.dma_start(wv_sb, wv.ap().rearrange("(k p) n -> p k n", p=P))
        wo_sb = singles.tile([D, H, C], bf16)
        nc.sync.dma_start(wo_sb, wo.ap().rearrange("(h d) n -> d h n", d=D))
        bo_sb = singles.tile([P, KC], f32)
        nc.sync.dma_start(bo_sb, bo.ap().rearrange("(k p) -> p k", p=P))

        # ---- projection outputs ----
        qt_sb = singles.tile([D, H, TOK], bf16)
        kt_sb = singles.tile([D, H, TOK], bf16)
        ktt_sb = singles.tile([D, H, NSEL], bf16)
        vown = singles.tile([P, BL * NKO, H, DA], bf16)
        vtok = singles.tile([P, NKS, H, DA], bf16)
        nrm = singles.tile([D, BL, H, N], bf16, tag="xst_nrm")

        def mm_cols(psum, lhsT, rhs_fn, start, stop, width):
            # split free dim into <=512 chunks, each within one PSUM bank
            c0 = 0
            while c0 < width:
                cw = min(512, width - c0)
                nc.tensor.matmul(
                    psum[:, c0 : c0 + cw],
                    lhsT,
                    rhs_fn(c0, cw),
                    start=start,
                    stop=stop,
                )
                c0 += cw

        # ---- phase 1a: qT / kT / kT_tok  ([d, tok] layouts) ----
        for w_sb, dst, src, width in (
            (wq_sb, qt_sb, xt_sb, TOK),
            (wk_sb, kt_sb, xt_sb, TOK),
            (wk_sb, ktt_sb, xst_sb, NSEL),
        ):
            for h in range(H):
                t0 = 0
                while t0 < width:
                    tw = min(N, width - t0)
                    psum = ppsum.tile([P, N], f32, tag="sc")
                    for k in range(KC):
                        mm_cols(
                            psum[0:D, :],
                            w_sb[:, k, h * D : (h + 1) * D],
                            lambda c0, cw, _k=k, _t0=t0: src[
                                :, _k, _t0 + c0 : _t0 + c0 + cw
                            ],
                            start=(k == 0),
                            stop=(k == KC - 1),
                            width=tw,
                        )
                    nc.any.tensor_copy(dst[0:D, h, t0 : t0 + tw], psum[0:D, 0:tw])
                    t0 += tw

        # ---- phase 1b: v / v_tok  ([tok, c] layout, head-strided + ones col) ----
        for src, nk, dstv in ((xt_sb, BL * NKO, vown), (xst_sb, NKS, vtok)):
            for kv in range(nk):
                psum = ppsum.tile([P, N], f32, tag="sc")
                for k in range(KC):
                    mm_cols(
                        psum,
                        src[:, k, kv * P : (kv + 1) * P],
                        lambda c0, cw, _k=k: wv_sb[:, _k, c0 : c0 + cw],
                        start=(k == 0),
                        stop=(k == KC - 1),
                        width=C,
                    )
                nc.any.tensor_copy(
                    dstv[:, kv, :, 0:D], psum.rearrange("p (h d) -> p h d", h=H)
                )
            nc.vector.memset(dstv[:, :, :, D:DNM], 0.0)
            nc.vector.memset(dstv[:, :, :, DNM:DA], 1.0)

        # ---- phase 2: attention per (frame, head) ----
        for f in range(BL):
            for h in range(H):
                pv = qpsum.tile([P, N], f32, tag="pv")
                for kt in range(NKV):
                    if kt < NKO:
                        ksrc = kt_sb[0:D, h, f * N + kt * P : f * N + (kt + 1) * P]
                        vsrc = vown[:, f * NKO + kt, h, :]
                    else:
                        ksrc = ktt_sb[0:D, h, (kt - NKO) * P : (kt - NKO + 1) * P]
                        vsrc = vtok[:, kt - NKO, h, :]
                    sc = ppsum.tile([P, N], f32, tag="sc")
                    mm_cols(
                        sc,
                        ksrc,
                        lambda c0, cw: qt_sb[0:D, h, f * N + c0 : f * N + c0 + cw],
                        start=True,
                        stop=True,
                        width=N,
                    )
                    e = expp.tile([P, N], bf16)
                    nc.scalar.activation(e, sc, Exp)
                    mm_cols(
                        pv[0:DA, :],
                        vsrc,
                        lambda c0, cw, _e=e: _e[:, c0 : c0 + cw],
                        start=(kt == 0),
                        stop=(kt == NKV - 1),
                        width=N,
                    )
                recip = rp.tile([1, N], f32, tag="recip", bufs=2)
                nc.vector.reciprocal(recip, pv[DNM : DNM + 1, :])
                recipb = rp.tile([D, N], f32, tag="recipb", bufs=2)
                nc.gpsimd.partition_broadcast(recipb, recip)
                nc.vector.tensor_mul(nrm[0:D, f, h, :], pv[0:D, :], recipb)

        # ---- phase 3: output projection + bias ----
        for f in range(BL):
            for m in range(KC):
                fp = qpsum.tile([P, N], f32, tag="pv")
                for h in range(H):
                    mm_cols(
                        fp,
                        wo_sb[0:D, h, m * P : (m + 1) * P],
                        lambda c0, cw, _f=f, _h=h: nrm[0:D, _f, _h, c0 : c0 + cw],
                        start=(h == 0),
                        stop=(h == H - 1),
                        width=N,
                    )
                o = ob.tile([P, N], f32)
                nc.vector.tensor_scalar_add(o, fp, bo_sb[:, m : m + 1])
                nc.sync.dma_start(
                    outt.ap()[m * P : (m + 1) * P, f * N : (f + 1) * N], o
                )


def _host_prep(x, Wq, Wk, Wv, Wo, bo, sacfa_mask, n_sel):
    """Shard + pre-layout inputs on the host (data movement / casts only)."""
    n_sel = int(n_sel)
    assert n_sel == NSEL, f"kernel hardcodes n_sel={NSEL}, got {n_sel}"
    x = np.asarray(x, np.float32)
    x_flat = x.reshape(B * N, C)

    # replicate jnp.nonzero(mask > 0.5, size=n_sel)[0]: first n_sel hits, 0-padded
    idx = np.flatnonzero(np.asarray(sacfa_mask) > 0.5)
    sel = np.zeros(NSEL, np.int64)
    m = min(NSEL, idx.size)
    sel[:m] = idx[:m]

    xsel_t = np.ascontiguousarray(x_flat[sel].T).astype(_BF16)  # [C, NSEL]
    scale = float(D) ** -0.5
    wq_b = (np.asarray(Wq, np.float32) * scale).astype(_BF16)
    wk_b = np.asarray(Wk, np.float32).astype(_BF16)
    wv_b = np.asarray(Wv, np.float32).astype(_BF16)
    wo_b = np.asarray(Wo, np.float32).astype(_BF16)
    bo_f = np.ascontiguousarray(np.asarray(bo, np.float32))

    in_maps = []
    for core in range(NCORES):
        xl = x[core * BL : (core + 1) * BL].reshape(TOK, C)
        in_maps.append(
            {
                "xt": np.ascontiguousarray(xl.T).astype(_BF16),
                "xst": xsel_t,
                "wq": wq_b,
                "wk": wk_b,
                "wv": wv_b,
                "wo": wo_b,
                "bo": bo_f,
            }
        )
    return in_maps


_CACHED_NC = None


def _get_nc():
    global _CACHED_NC
    if _CACHED_NC is None:
        _CACHED_NC = _build_bass()
    return _CACHED_NC


def kernel(x, Wq, Wk, Wv, Wo, bo, sacfa_mask, n_sel, _trace=False):
    from concourse import bass_utils

    in_maps = _host_prep(x, Wq, Wk, Wv, Wo, bo, sacfa_mask, n_sel)
    nc = _get_nc()
    res = bass_utils.run_bass_kernel_spmd(
        nc, in_maps, core_ids=list(range(NCORES)), trace=_trace
    )
    out = np.empty((B, N, C), np.float32)
    for core in range(NCORES):
        ot = res.results[core]["outt"]  # [C, TOK] f32
        out[core * BL : (core + 1) * BL] = ot.T.reshape(BL, N, C)
    if _trace:
        kernel.last_results = res
    return out
